# revision 1
# baseline (speedup 1.0000x reference)
"""Trainium2 Bass kernel for loopy-BP GNN message passing (8 NeuronCores).

Undirected pairs sharded across 8 cores (pair i -> core i%8). Each pair-slot
holds BOTH directed messages, so reverse-message access is slot-local (no
permutation). Pairs grouped into 16 (u-window, v-window) sections so every
dma_gather / dma_scatter_add uses int16 window-local indices; within each
section pairs are greedily edge-colored so each scatter call has distinct
target rows (CCE add is not duplicate-safe). Node tables are 256B-pitched
for the 256B-elem gather; node space uses a windowed row map with per-window
pad rows that serve as trash targets. Per iteration: gather log-beliefs of
both endpoints, compute both directed messages, scatter-add log-messages
into the pitched per-node sum table, ReduceScatter + node update + AllGather.
"""
import numpy as np

NCORES = 8
S = 16
EPS_POT = 1.0
DIFFUSION = 3
A_COEF = float((np.exp(EPS_POT) - 1.0) / (np.exp(EPS_POT) + 15.0))
B_COEF = float(1.0 / (np.exp(EPS_POT) + 15.0))
NWIN = 4
CALL_ROWS = 1024

_CACHE = {}


def _round_up(x, m):
    return -(-x // m) * m


def _geom(n_nodes):
    win_real = -(-n_nodes // NWIN)
    win_pad = _round_up(win_real + 64, 256)
    npad = NWIN * win_pad
    return win_real, win_pad, npad


def _plan(u, v, n_nodes):
    win_real, win_pad, npad = _geom(n_nodes)
    per_core = []
    max_class = {}
    for c in range(NCORES):
        sel = np.where(np.arange(u.shape[0]) % NCORES == c)[0]
        uu, vv = u[sel], v[sel]
        sec = (uu // win_real) * NWIN + (vv // win_real)
        order = np.argsort(sec * (n_nodes + 1) + uu, kind="stable")
        uu, vv, sec = uu[order], vv[order], sec[order]
        color = np.zeros(len(uu), np.int32)
        ucol, vcol = {}, {}
        for i in range(len(uu)):
            ks = int(sec[i])
            cu = ucol.setdefault((ks, int(uu[i])), set())
            cv = vcol.setdefault((ks, int(vv[i])), set())
            k = 0
            while k in cu or k in cv:
                k += 1
            color[i] = k
            cu.add(k)
            cv.add(k)
        per_core.append((uu, vv, sec, color))
        keys, cnts = np.unique(sec.astype(np.int64) * 1000 + color, return_counts=True)
        for kk, cc in zip(keys, cnts):
            max_class[int(kk)] = max(max_class.get(int(kk), 0), int(cc))

    class_keys = sorted(max_class)
    class_size = {k: _round_up(max_class[k], 128) for k in class_keys}
    total = sum(class_size.values())

    calls = []
    ofs = 0
    for k in class_keys:
        sz = class_size[k]
        p = 0
        while p < sz:
            n = min(CALL_ROWS, sz - p)
            calls.append((ofs + p, n, k // 1000))
            p += n
        ofs += sz

    TRASH = win_real  # window-local trash row (per-window pad region)
    ug16 = np.full((NCORES, total), 0, np.int16)
    vg16 = np.full((NCORES, total), 0, np.int16)
    us16 = np.full((NCORES, total), TRASH, np.int16)
    vs16 = np.full((NCORES, total), TRASH, np.int16)
    for c in range(NCORES):
        uu, vv, sec, color = per_core[c]
        keys = sec.astype(np.int64) * 1000 + color
        order = np.argsort(keys * (n_nodes + 1) + uu, kind="stable")
        base = {}
        ofs = 0
        for k in class_keys:
            base[k] = ofs
            ofs += class_size[k]
        cur = dict.fromkeys(class_keys, 0)
        pos = np.zeros(len(uu), np.int64)
        for i in order:
            k = int(keys[i])
            pos[i] = base[k] + cur[k]
            cur[k] += 1
        ul = (uu % win_real).astype(np.int16)
        vl = (vv % win_real).astype(np.int16)
        ug16[c, pos] = ul
        vg16[c, pos] = vl
        us16[c, pos] = ul
        vs16[c, pos] = vl
    return dict(calls=calls, total=total, ug16=ug16, vg16=vg16,
                us16=us16, vs16=vs16, win_pad=win_pad, win_real=win_real,
                npad=npad)


def _wrap16(a):
    n = a.shape[0]
    out = np.zeros((128, n // 16), np.int16)
    blk = a.reshape(n // 16, 16).T
    for g in range(8):
        out[g * 16:(g + 1) * 16] = blk
    return out


def _build(plan, n_nodes, feat_dim):
    import concourse.bacc as bacc
    import concourse.tile as tile
    import concourse.mybir as mybir
    from concourse import library_config
    from concourse.masks import make_identity

    dt = mybir.dt
    AF = mybir.ActivationFunctionType
    AL = mybir.AluOpType
    AX = mybir.AxisListType
    total = plan["total"]
    calls = plan["calls"]
    win = plan["win_pad"]
    npad = plan["npad"]
    cols = total // 128
    shard = npad // NCORES
    nblk = shard // 128
    rg = [list(range(NCORES))]

    nc = bacc.Bacc("TRN2", target_bir_lowering=False, debug=False,
                   num_devices=NCORES, num_swdge_queues=4)

    feat_in = nc.dram_tensor("feat", [shard, feat_dim], dt.float32, kind="ExternalInput")
    w_in = nc.dram_tensor("wmat", [feat_dim, S], dt.float32, kind="ExternalInput")
    us_in = nc.dram_tensor("us", [128, total // 16], dt.int16, kind="ExternalInput")
    vs_in = nc.dram_tensor("vs", [128, total // 16], dt.int16, kind="ExternalInput")
    priors_out = nc.dram_tensor("priors", [shard, S], dt.float32, kind="ExternalOutput")
    beliefs_out = nc.dram_tensor("beliefs", [shard, S], dt.float32, kind="ExternalOutput")

    logb_tab = nc.dram_tensor("logb_tab", [npad, 64], dt.float32)
    s_tab = nc.dram_tensor("s_tab", [npad, 64], dt.float32)
    l_tab0 = nc.dram_tensor("l_tab0", [128, cols * 16], dt.float32)
    l_tab1 = nc.dram_tensor("l_tab1", [128, cols * 16], dt.float32)
    rs_in = nc.dram_tensor("rs_in", [npad, S], dt.float32)
    rs_out = nc.dram_tensor("rs_out", [shard, S], dt.float32)
    ag_in = nc.dram_tensor("ag_in", [shard, S], dt.float32)
    ag_out = nc.dram_tensor("ag_out", [npad, S], dt.float32, addr_space="Shared")

    qn = [0]

    def nq():
        qn[0] = (qn[0] + 1) % 4
        return 0  # TODO: multi-queue once Tile sem assignment supports it

    with tile.TileContext(nc) as tc:
        with tc.tile_pool(name="const", bufs=1) as cpool, \
             tc.tile_pool(name="sbuf", bufs=3) as pool, \
             tc.tile_pool(name="node", bufs=1) as npool, \
             tc.tile_pool(name="bigb", bufs=2) as bpool, \
             tc.tile_pool(name="psum", bufs=2, space="PSUM") as pp:
            nc.gpsimd.load_library(library_config.mlp)
            bconst = nc.alloc_sbuf_tensor("bconst", [128, 1], dt.float32)
            nc.gpsimd.memset(bconst.ap(), B_COEF)
            nc.const_aps.aps[(dt.float32, B_COEF)] = bconst.ap()
            ident = cpool.tile([128, 128], dt.float32)
            make_identity(nc, ident[:])
            wt = cpool.tile([128, S], dt.float32)
            nc.sync.dma_start(wt[:], w_in[:])
            us_t = cpool.tile([128, total // 16], dt.int16)
            nc.sync.dma_start(us_t[:], us_in[:])
            vs_t = cpool.tile([128, total // 16], dt.int16)
            nc.sync.dma_start(vs_t[:], vs_in[:])

            # ---- priors ----
            logp = cpool.tile([128, nblk, S], dt.float32)
            for b in range(nblk):
                ft = pool.tile([128, feat_dim], dt.float32, tag="ft")
                nc.sync.dma_start(ft[:], feat_in[b * 128:(b + 1) * 128, :])
                ps_t = pp.tile([128, 128], dt.float32, tag="ps_t")
                nc.tensor.transpose(out=ps_t[:, 0:feat_dim], in_=ft[:], identity=ident[:])
                ftT = pool.tile([128, 128], dt.float32, tag="ftT")
                nc.vector.tensor_copy(out=ftT[:], in_=ps_t[:])
                ps_l = pp.tile([128, S], dt.float32, tag="ps_l")
                nc.tensor.matmul(ps_l[:], lhsT=ftT[:, 0:128], rhs=wt[:], start=True, stop=True)
                mx = pool.tile([128, 1], dt.float32, tag="mx")
                nc.vector.tensor_reduce(mx[:], ps_l[:], axis=AX.X, op=AL.max)
                lg = pool.tile([128, S], dt.float32, tag="lg")
                nc.vector.scalar_tensor_tensor(lg[:], in0=ps_l[:], scalar=1.0,
                                               in1=mx[:].to_broadcast([128, S]),
                                               op0=AL.mult, op1=AL.subtract)
                ex = pool.tile([128, S], dt.float32, tag="ex")
                nc.scalar.activation(ex[:], lg[:], AF.Exp)
                sm = pool.tile([128, 1], dt.float32, tag="sm")
                nc.vector.tensor_reduce(sm[:], ex[:], axis=AX.X, op=AL.add)
                rc = pool.tile([128, 1], dt.float32, tag="rc")
                nc.vector.reciprocal(rc[:], sm[:])
                pr = pool.tile([128, S], dt.float32, tag="pr")
                nc.vector.tensor_tensor(pr[:], ex[:], rc[:].to_broadcast([128, S]), op=AL.mult)
                nc.sync.dma_start(priors_out[b * 128:(b + 1) * 128, :], pr[:])
                nc.scalar.activation(logp[:, b, :], pr[:], AF.Ln)

            logb_sh = cpool.tile([128, nblk, S], dt.float32)
            mx0 = npool.tile([128, nblk], dt.float32, tag="mx0")
            nc.vector.tensor_reduce(mx0[:], logp[:], axis=AX.X, op=AL.max)
            nc.vector.scalar_tensor_tensor(
                logb_sh[:], in0=logp[:], scalar=1.0,
                in1=mx0[:].rearrange("p (b o) -> p b o", o=1).to_broadcast([128, nblk, S]),
                op0=AL.mult, op1=AL.subtract)
            nc.sync.dma_start(ag_in[:].rearrange("(b p) s -> p b s", p=128), logb_sh[:])
            nc.gpsimd.collective_compute("AllGather", AL.bypass, replica_groups=rg,
                                         ins=[ag_in[:]], outs=[ag_out[:]])

            CH = 28
            for it in range(1, DIFFUSION + 1):
                # pitched logb table from ag_out
                for b0 in range(0, npad // 128, CH):
                    bn = min(CH, npad // 128 - b0)
                    cm = bpool.tile([128, CH, S], dt.float32, tag="cm")
                    nc.sync.dma_start(
                        cm[:, :bn, :],
                        ag_out[:].rearrange("(b p) s -> p b s", p=128)[:, b0:b0 + bn, :])
                    pit = bpool.tile([128, CH, 64], dt.float32, tag="pit")
                    nc.vector.memset(pit[:], 0.0)
                    nc.vector.tensor_copy(out=pit[:, :bn, 0:S], in_=cm[:, :bn, :])
                    nc.sync.dma_start(
                        logb_tab[:].rearrange("(b p) c -> p b c", p=128)[:, b0:b0 + bn, :],
                        pit[:, :bn, :])
                zt = bpool.tile([128, CH, 64], dt.float32, tag="zt")
                nc.vector.memset(zt[:], 0.0)
                for b0 in range(0, npad // 128, CH):
                    bn = min(CH, npad // 128 - b0)
                    nc.sync.dma_start(
                        s_tab[:].rearrange("(b p) c -> p b c", p=128)[:, b0:b0 + bn, :],
                        zt[:, :bn, :])

                for (ofs, n, sec) in calls:
                    ncol = n // 128
                    c0 = ofs // 128
                    uw, vw = sec // NWIN, sec % NWIN
                    i0, i1 = ofs // 16, (ofs + n) // 16
                    gu = pool.tile([128, ncol, 64], dt.float32, tag="gu")
                    nc.gpsimd.dma_gather(
                        out_ap=gu[:, :ncol, :], in_ap=logb_tab[uw * win:(uw + 1) * win, :],
                        idxs_ap=us_t[:, i0:i1], num_idxs=n, num_idxs_reg=n,
                        elem_size=64, queue_num=nq())
                    gv = pool.tile([128, ncol, 64], dt.float32, tag="gv")
                    nc.gpsimd.dma_gather(
                        out_ap=gv[:, :ncol, :], in_ap=logb_tab[vw * win:(vw + 1) * win, :],
                        idxs_ap=vs_t[:, i0:i1], num_idxs=n, num_idxs_reg=n,
                        elem_size=64, queue_num=nq())
                    lms = [None, None]
                    if it > 1:
                        for d, ltab in enumerate([l_tab1, l_tab0]):
                            lm = pool.tile([128, ncol, S], dt.float32, tag=f"lm{d}")
                            nc.sync.dma_start(
                                lm[:], ltab[:, c0 * 16:(c0 + ncol) * 16]
                                .rearrange("p (a s) -> p a s", s=S))
                            lms[d] = lm
                    lgms = []
                    for d, gx in enumerate([gu, gv]):
                        tt = pool.tile([128, ncol, S], dt.float32, tag=f"tt{d}")
                        if it > 1:
                            nc.vector.scalar_tensor_tensor(
                                tt[:], in0=lms[d][:], scalar=-1.0,
                                in1=gx[:, :ncol, 0:S], op0=AL.mult, op1=AL.add)
                        else:
                            nc.vector.tensor_copy(out=tt[:], in_=gx[:, :ncol, 0:S])
                        rr = pool.tile([128, ncol, S], dt.float32, tag=f"rr{d}")
                        nc.scalar.activation(rr[:], tt[:], AF.Exp)
                        rsum = pool.tile([128, ncol], dt.float32, tag=f"rsum{d}")
                        nc.vector.tensor_reduce(rsum[:], rr[:], axis=AX.X, op=AL.add)
                        rcp = pool.tile([128, ncol], dt.float32, tag=f"rcp{d}")
                        nc.vector.reciprocal(rcp[:], rsum[:])
                        nm = pool.tile([128, ncol, S], dt.float32, tag=f"nm{d}")
                        nc.vector.tensor_tensor(
                            nm[:], rr[:],
                            rcp[:].rearrange("p (a o) -> p a o", o=1).to_broadcast([128, ncol, S]),
                            op=AL.mult)
                        lgm = pool.tile([128, ncol, S], dt.float32, tag=f"lgm{d}")
                        nc.scalar.activation(lgm[:], nm[:], AF.Ln, bias=B_COEF, scale=A_COEF)
                        outtab = l_tab0 if d == 0 else l_tab1
                        nc.sync.dma_start(
                            outtab[:, c0 * 16:(c0 + ncol) * 16],
                            lgm[:].rearrange("p a s -> p (a s)"))
                        lgms.append(lgm)
                    nc.gpsimd.dma_scatter_add(
                        out_ap=s_tab[vw * win:, 0:S], in_ap=lgms[0][:],
                        idxs_ap=vs_t[:, i0:i1], num_idxs=n, num_idxs_reg=n,
                        elem_size=S, elem_step=64, queue_num=nq())
                    nc.gpsimd.dma_scatter_add(
                        out_ap=s_tab[uw * win:, 0:S], in_ap=lgms[1][:],
                        idxs_ap=us_t[:, i0:i1], num_idxs=n, num_idxs_reg=n,
                        elem_size=S, elem_step=64, queue_num=nq())

                for b0 in range(0, npad // 128, CH):
                    bn = min(CH, npad // 128 - b0)
                    pit2 = bpool.tile([128, CH, 64], dt.float32, tag="pit2")
                    nc.sync.dma_start(
                        pit2[:, :bn, :],
                        s_tab[:].rearrange("(b p) c -> p b c", p=128)[:, b0:b0 + bn, :])
                    cm2 = bpool.tile([128, CH, S], dt.float32, tag="cm2")
                    nc.vector.tensor_copy(out=cm2[:, :bn, :], in_=pit2[:, :bn, 0:S])
                    nc.sync.dma_start(
                        rs_in[:].rearrange("(b p) s -> p b s", p=128)[:, b0:b0 + bn, :],
                        cm2[:, :bn, :])
                nc.gpsimd.collective_compute("ReduceScatter", AL.add, replica_groups=rg,
                                             ins=[rs_in[:]], outs=[rs_out[:]])
                sv = npool.tile([128, nblk, S], dt.float32, tag="sv")
                nc.sync.dma_start(sv[:], rs_out[:].rearrange("(b p) s -> p b s", p=128))
                lb = npool.tile([128, nblk, S], dt.float32, tag="lb")
                nc.vector.tensor_tensor(lb[:], logp[:], sv[:], op=AL.add)
                mxi = npool.tile([128, nblk], dt.float32, tag="mxi")
                nc.vector.tensor_reduce(mxi[:], lb[:], axis=AX.X, op=AL.max)
                lbn = npool.tile([128, nblk, S], dt.float32, tag="lbn")
                nc.vector.scalar_tensor_tensor(
                    lbn[:], in0=lb[:], scalar=1.0,
                    in1=mxi[:].rearrange("p (b o) -> p b o", o=1).to_broadcast([128, nblk, S]),
                    op0=AL.mult, op1=AL.subtract)
                if it < DIFFUSION:
                    nc.sync.dma_start(ag_in[:].rearrange("(b p) s -> p b s", p=128), lbn[:])
                    nc.gpsimd.collective_compute("AllGather", AL.bypass, replica_groups=rg,
                                                 ins=[ag_in[:]], outs=[ag_out[:]])
                else:
                    eb = npool.tile([128, nblk, S], dt.float32, tag="eb")
                    nc.scalar.activation(eb[:], lbn[:], AF.Exp)
                    sb = npool.tile([128, nblk], dt.float32, tag="sb")
                    nc.vector.tensor_reduce(sb[:], eb[:], axis=AX.X, op=AL.add)
                    rb = npool.tile([128, nblk], dt.float32, tag="rb")
                    nc.vector.reciprocal(rb[:], sb[:])
                    bf = npool.tile([128, nblk, S], dt.float32, tag="bf")
                    nc.vector.tensor_tensor(
                        bf[:], eb[:],
                        rb[:].rearrange("p (b o) -> p b o", o=1).to_broadcast([128, nblk, S]),
                        op=AL.mult)
                    nc.sync.dma_start(beliefs_out[:].rearrange("(b p) s -> p b s", p=128), bf[:])
    nc.compile()
    return nc


def kernel(features, W, src_nodes, dst_nodes, rev_edges):
    import concourse.bass_utils as bass_utils

    features = np.asarray(features, np.float32)
    W = np.asarray(W, np.float32)
    src = np.asarray(src_nodes, np.int64)
    dst = np.asarray(dst_nodes, np.int64)
    rev = np.asarray(rev_edges, np.int64)
    n_nodes, feat_dim = features.shape
    E = src.shape[0] // 2
    assert np.array_equal(rev[:E], np.arange(E) + E) and \
        np.array_equal(rev[E:], np.arange(E)), "unexpected rev_edges structure"
    u = src[:E].astype(np.int64)
    v = dst[:E].astype(np.int64)

    key = (n_nodes, feat_dim, E)
    if key not in _CACHE:
        plan = _plan(u, v, n_nodes)
        nc = _build(plan, n_nodes, feat_dim)
        _CACHE[key] = (plan, nc)
    plan, nc = _CACHE[key]

    win_real, win_pad, npad = _geom(n_nodes)
    rowmap = (np.arange(n_nodes) // win_real) * win_pad + np.arange(n_nodes) % win_real
    featpad = np.zeros((npad, feat_dim), np.float32)
    featpad[rowmap] = features
    shard = npad // NCORES
    in_maps = []
    for c in range(NCORES):
        in_maps.append({
            "feat": np.ascontiguousarray(featpad[c * shard:(c + 1) * shard]),
            "wmat": W,
            "us": _wrap16(plan["us16"][c]),
            "vs": _wrap16(plan["vs16"][c]),
        })
    res = bass_utils.run_bass_kernel_spmd(nc, in_maps, core_ids=list(range(NCORES)))
    priors_pad = np.concatenate([res.results[c]["priors"] for c in range(NCORES)], 0)
    beliefs_pad = np.concatenate([res.results[c]["beliefs"] for c in range(NCORES)], 0)
    return priors_pad[rowmap], beliefs_pad[rowmap]



# revision 14
# speedup vs baseline: 6.5794x; 6.5794x over previous
"""Trainium2 Bass kernel for loopy-BP GNN message passing (8 NeuronCores).

Undirected pairs sharded across 8 cores (pair i -> core i%8). Each pair-slot
holds BOTH directed messages, so reverse-message access is slot-local (no
permutation). Pairs grouped into 16 (u-window, v-window) sections so every
dma_gather / dma_scatter_add uses int16 window-local indices; within each
section pairs are greedily edge-colored so each scatter call has distinct
target rows (CCE add is not duplicate-safe). Node tables are 256B-pitched
for the 256B-elem gather; node space uses a windowed row map with per-window
pad rows that serve as trash targets. Per iteration: gather log-beliefs of
both endpoints, compute both directed messages, scatter-add log-messages
into the pitched per-node sum table, ReduceScatter + node update + AllGather.

Host<->device I/O is minimized for the axon tunnel (~80 MB/s): all inputs are
packed into one int16 blob per core (features as fp16 bits, idx tables
unreplicated 16-partition form, W as fp16 bits), outputs are one fp16
[shard, 32] tensor (priors | beliefs), and the compiled PJRT executable is
cached so repeat calls skip trace/lower/compile.
"""
import numpy as np

NCORES = 8
S = 16
EPS_POT = 1.0
DIFFUSION = 3
A_COEF = float((np.exp(EPS_POT) - 1.0) / (np.exp(EPS_POT) + 15.0))
B_COEF = float(1.0 / (np.exp(EPS_POT) + 15.0))
NWIN = 4
CALL_ROWS = 1024

_CACHE = {}


def _round_up(x, m):
    return -(-x // m) * m


def _geom(n_nodes):
    win_real = -(-n_nodes // NWIN)
    win_pad = _round_up(win_real + 64, 256)
    npad = NWIN * win_pad
    return win_real, win_pad, npad


def _plan(u, v, n_nodes):
    win_real, win_pad, npad = _geom(n_nodes)
    per_core = []
    max_class = {}
    for c in range(NCORES):
        sel = np.where(np.arange(u.shape[0]) % NCORES == c)[0]
        uu, vv = u[sel], v[sel]
        sec = (uu // win_real) * NWIN + (vv // win_real)
        order = np.argsort(sec * (n_nodes + 1) + uu, kind="stable")
        uu, vv, sec = uu[order], vv[order], sec[order]
        color = np.zeros(len(uu), np.int32)
        ucol, vcol = {}, {}
        for i in range(len(uu)):
            ks = int(sec[i])
            cu = ucol.setdefault((ks, int(uu[i])), set())
            cv = vcol.setdefault((ks, int(vv[i])), set())
            k = 0
            while k in cu or k in cv:
                k += 1
            color[i] = k
            cu.add(k)
            cv.add(k)
        per_core.append((uu, vv, sec, color))
        keys, cnts = np.unique(sec.astype(np.int64) * 1000 + color, return_counts=True)
        for kk, cc in zip(keys, cnts):
            max_class[int(kk)] = max(max_class.get(int(kk), 0), int(cc))

    class_keys = sorted(max_class)
    class_size = {k: _round_up(max_class[k], 128) for k in class_keys}
    total = sum(class_size.values())

    calls = []
    ofs = 0
    for k in class_keys:
        sz = class_size[k]
        p = 0
        while p < sz:
            n = min(CALL_ROWS, sz - p)
            calls.append((ofs + p, n, k // 1000))
            p += n
        ofs += sz

    TRASH = win_real  # window-local trash row (per-window pad region)
    us16 = np.full((NCORES, total), TRASH, np.int16)
    vs16 = np.full((NCORES, total), TRASH, np.int16)
    for c in range(NCORES):
        uu, vv, sec, color = per_core[c]
        keys = sec.astype(np.int64) * 1000 + color
        order = np.argsort(keys * (n_nodes + 1) + uu, kind="stable")
        base = {}
        ofs = 0
        for k in class_keys:
            base[k] = ofs
            ofs += class_size[k]
        cur = dict.fromkeys(class_keys, 0)
        pos = np.zeros(len(uu), np.int64)
        for i in order:
            k = int(keys[i])
            pos[i] = base[k] + cur[k]
            cur[k] += 1
        us16[c, pos] = (uu % win_real).astype(np.int16)
        vs16[c, pos] = (vv % win_real).astype(np.int16)
    # 16-partition wrapped layout, flattened: (16, total//16) row-major
    us_wrap = np.ascontiguousarray(
        us16.reshape(NCORES, total // 16, 16).transpose(0, 2, 1)
    ).reshape(NCORES, total)
    vs_wrap = np.ascontiguousarray(
        vs16.reshape(NCORES, total // 16, 16).transpose(0, 2, 1)
    ).reshape(NCORES, total)
    return dict(calls=calls, total=total, us_wrap=us_wrap, vs_wrap=vs_wrap,
                win_pad=win_pad, win_real=win_real, npad=npad)


def _blob_layout(plan, n_nodes, feat_dim):
    _, _, npad = _geom(n_nodes)
    shard = npad // NCORES
    total = plan["total"]
    feat_elems = shard * feat_dim
    w_elems = feat_dim * S
    # [feat fp16 | us int16 | vs int16 | W fp16], all 2-byte elems
    off_us = feat_elems
    off_vs = off_us + total
    off_w = off_vs + total
    nelem = off_w + w_elems
    return shard, feat_elems, off_us, off_vs, off_w, nelem


def _build(plan, n_nodes, feat_dim):
    import concourse.bacc as bacc
    import concourse.tile as tile
    import concourse.mybir as mybir
    from concourse import library_config
    from concourse.masks import make_identity

    dt = mybir.dt
    AF = mybir.ActivationFunctionType
    AL = mybir.AluOpType
    AX = mybir.AxisListType
    total = plan["total"]
    calls = plan["calls"]
    win = plan["win_pad"]
    npad = plan["npad"]
    shard, feat_elems, off_us, off_vs, off_w, nelem = _blob_layout(
        plan, n_nodes, feat_dim)
    nblk = shard // 128
    CW = total // 16
    rg = [list(range(NCORES))]

    nc = bacc.Bacc("TRN2", target_bir_lowering=False, debug=False,
                   num_devices=NCORES, num_swdge_queues=4)

    blob = nc.dram_tensor("blob", [1, nelem], dt.int16, kind="ExternalInput")
    out16 = nc.dram_tensor("out16", [shard, 2 * S], dt.float16,
                           kind="ExternalOutput")

    logb_tab = nc.dram_tensor("logb_tab", [npad, 64], dt.float32)
    s_tab = nc.dram_tensor("s_tab", [npad, 64], dt.float32)
    l_tab0 = nc.dram_tensor("l_tab0", [128, (total // 128) * 16], dt.float32)
    l_tab1 = nc.dram_tensor("l_tab1", [128, (total // 128) * 16], dt.float32)
    rs_in = nc.dram_tensor("rs_in", [npad, S], dt.float32)
    rs_out = nc.dram_tensor("rs_out", [shard, S], dt.float32)
    ag_in = nc.dram_tensor("ag_in", [shard, S], dt.float32)
    ag_out = nc.dram_tensor("ag_out", [npad, S], dt.float32, addr_space="Shared")

    blob_feat = blob[:, 0:feat_elems].bitcast(dt.float16).rearrange(
        "x (b p c) -> (x b) p c", p=128, c=feat_dim)
    blob_us = blob[:, off_us:off_us + total].rearrange(
        "x (p c) -> (x p) c", p=16)
    blob_vs = blob[:, off_vs:off_vs + total].rearrange(
        "x (p c) -> (x p) c", p=16)
    blob_w = blob[:, off_w:off_w + feat_dim * S].bitcast(dt.float16).rearrange(
        "x (p c) -> (x p) c", p=feat_dim)

    with tile.TileContext(nc) as tc:
        with tc.tile_pool(name="const", bufs=1) as cpool, \
             tc.tile_pool(name="sbuf", bufs=3) as pool, \
             tc.tile_pool(name="node", bufs=1) as npool, \
             tc.tile_pool(name="bigb", bufs=2) as bpool, \
             tc.tile_pool(name="psum", bufs=2, space="PSUM") as pp:
            nc.gpsimd.load_library(library_config.mlp)
            bconst = nc.alloc_sbuf_tensor("bconst", [128, 1], dt.float32)
            nc.gpsimd.memset(bconst.ap(), B_COEF)
            nc.const_aps.aps[(dt.float32, B_COEF)] = bconst.ap()
            ident = cpool.tile([128, 128], dt.float32)
            make_identity(nc, ident[:])
            wt16 = cpool.tile([128, S], dt.float16)
            nc.sync.dma_start(wt16[:], blob_w)
            wt = cpool.tile([128, S], dt.float32)
            nc.vector.tensor_copy(out=wt[:], in_=wt16[:])
            us_t = cpool.tile([128, CW], dt.int16)
            vs_t = cpool.tile([128, CW], dt.int16)
            for g in range(8):
                nc.sync.dma_start(us_t[16 * g:16 * (g + 1), :], blob_us)
                nc.sync.dma_start(vs_t[16 * g:16 * (g + 1), :], blob_vs)

            # ---- priors ----
            logp = cpool.tile([128, nblk, S], dt.float32)
            for b in range(nblk):
                ft16 = pool.tile([128, feat_dim], dt.float16, tag="ft16")
                nc.sync.dma_start(ft16[:], blob_feat[b, :, :])
                ft = pool.tile([128, feat_dim], dt.float32, tag="ft")
                nc.vector.tensor_copy(out=ft[:], in_=ft16[:])
                ps_t = pp.tile([128, 128], dt.float32, tag="ps_t")
                nc.tensor.transpose(out=ps_t[:, 0:feat_dim], in_=ft[:], identity=ident[:])
                ftT = pool.tile([128, 128], dt.float32, tag="ftT")
                nc.vector.tensor_copy(out=ftT[:], in_=ps_t[:])
                ps_l = pp.tile([128, S], dt.float32, tag="ps_l")
                nc.tensor.matmul(ps_l[:], lhsT=ftT[:, 0:128], rhs=wt[:], start=True, stop=True)
                mx = pool.tile([128, 1], dt.float32, tag="mx")
                nc.vector.tensor_reduce(mx[:], ps_l[:], axis=AX.X, op=AL.max)
                lg = pool.tile([128, S], dt.float32, tag="lg")
                nc.vector.scalar_tensor_tensor(lg[:], in0=ps_l[:], scalar=1.0,
                                               in1=mx[:].to_broadcast([128, S]),
                                               op0=AL.mult, op1=AL.subtract)
                ex = pool.tile([128, S], dt.float32, tag="ex")
                nc.scalar.activation(ex[:], lg[:], AF.Exp)
                sm = pool.tile([128, 1], dt.float32, tag="sm")
                nc.vector.tensor_reduce(sm[:], ex[:], axis=AX.X, op=AL.add)
                rc = pool.tile([128, 1], dt.float32, tag="rc")
                nc.vector.reciprocal(rc[:], sm[:])
                pr = pool.tile([128, S], dt.float32, tag="pr")
                nc.vector.tensor_tensor(pr[:], ex[:], rc[:].to_broadcast([128, S]), op=AL.mult)
                pr16 = pool.tile([128, S], dt.float16, tag="pr16")
                nc.vector.tensor_copy(out=pr16[:], in_=pr[:])
                nc.sync.dma_start(out16[b * 128:(b + 1) * 128, 0:S], pr16[:])
                nc.scalar.activation(logp[:, b, :], pr[:], AF.Ln)

            logb_sh = cpool.tile([128, nblk, S], dt.float32)
            mx0 = npool.tile([128, nblk], dt.float32, tag="mx0")
            nc.vector.tensor_reduce(mx0[:], logp[:], axis=AX.X, op=AL.max)
            nc.vector.scalar_tensor_tensor(
                logb_sh[:], in0=logp[:], scalar=1.0,
                in1=mx0[:].rearrange("p (b o) -> p b o", o=1).to_broadcast([128, nblk, S]),
                op0=AL.mult, op1=AL.subtract)
            nc.sync.dma_start(ag_in[:].rearrange("(b p) s -> p b s", p=128), logb_sh[:])
            nc.gpsimd.collective_compute("AllGather", AL.bypass, replica_groups=rg,
                                         ins=[ag_in[:]], outs=[ag_out[:]])

            CH = 24
            for it in range(1, DIFFUSION + 1):
                # pitched logb table from ag_out
                for b0 in range(0, npad // 128, CH):
                    bn = min(CH, npad // 128 - b0)
                    cm = bpool.tile([128, CH, S], dt.float32, tag="cm")
                    nc.sync.dma_start(
                        cm[:, :bn, :],
                        ag_out[:].rearrange("(b p) s -> p b s", p=128)[:, b0:b0 + bn, :])
                    pit = bpool.tile([128, CH, 64], dt.float32, tag="pit")
                    nc.vector.memset(pit[:], 0.0)
                    nc.vector.tensor_copy(out=pit[:, :bn, 0:S], in_=cm[:, :bn, :])
                    nc.sync.dma_start(
                        logb_tab[:].rearrange("(b p) c -> p b c", p=128)[:, b0:b0 + bn, :],
                        pit[:, :bn, :])
                zt = bpool.tile([128, CH, 64], dt.float32, tag="zt")
                nc.vector.memset(zt[:], 0.0)
                for b0 in range(0, npad // 128, CH):
                    bn = min(CH, npad // 128 - b0)
                    nc.sync.dma_start(
                        s_tab[:].rearrange("(b p) c -> p b c", p=128)[:, b0:b0 + bn, :],
                        zt[:, :bn, :])

                for (ofs, n, sec) in calls:
                    ncol = n // 128
                    c0 = ofs // 128
                    uw, vw = sec // NWIN, sec % NWIN
                    i0, i1 = ofs // 16, (ofs + n) // 16
                    gu = pool.tile([128, ncol, 64], dt.float32, tag="gu")
                    nc.gpsimd.dma_gather(
                        out_ap=gu[:, :ncol, :], in_ap=logb_tab[uw * win:(uw + 1) * win, :],
                        idxs_ap=us_t[:, i0:i1], num_idxs=n, num_idxs_reg=n,
                        elem_size=64, queue_num=0)
                    gv = pool.tile([128, ncol, 64], dt.float32, tag="gv")
                    nc.gpsimd.dma_gather(
                        out_ap=gv[:, :ncol, :], in_ap=logb_tab[vw * win:(vw + 1) * win, :],
                        idxs_ap=vs_t[:, i0:i1], num_idxs=n, num_idxs_reg=n,
                        elem_size=64, queue_num=0)
                    lms = [None, None]
                    if it > 1:
                        for d, ltab in enumerate([l_tab1, l_tab0]):
                            lm = pool.tile([128, ncol, S], dt.float32, tag=f"lm{d}")
                            nc.sync.dma_start(
                                lm[:], ltab[:, c0 * 16:(c0 + ncol) * 16]
                                .rearrange("p (a s) -> p a s", s=S))
                            lms[d] = lm
                    lgms = []
                    for d, gx in enumerate([gu, gv]):
                        tt = pool.tile([128, ncol, S], dt.float32, tag=f"tt{d}")
                        if it > 1:
                            nc.vector.scalar_tensor_tensor(
                                tt[:], in0=lms[d][:], scalar=-1.0,
                                in1=gx[:, :ncol, 0:S], op0=AL.mult, op1=AL.add)
                        else:
                            nc.vector.tensor_copy(out=tt[:], in_=gx[:, :ncol, 0:S])
                        rr = pool.tile([128, ncol, S], dt.float32, tag=f"rr{d}")
                        nc.scalar.activation(rr[:], tt[:], AF.Exp)
                        rsum = pool.tile([128, ncol], dt.float32, tag=f"rsum{d}")
                        nc.vector.tensor_reduce(rsum[:], rr[:], axis=AX.X, op=AL.add)
                        rcp = pool.tile([128, ncol], dt.float32, tag=f"rcp{d}")
                        nc.vector.reciprocal(rcp[:], rsum[:])
                        nm = pool.tile([128, ncol, S], dt.float32, tag=f"nm{d}")
                        nc.vector.tensor_tensor(
                            nm[:], rr[:],
                            rcp[:].rearrange("p (a o) -> p a o", o=1).to_broadcast([128, ncol, S]),
                            op=AL.mult)
                        lgm = pool.tile([128, ncol, S], dt.float32, tag=f"lgm{d}")
                        nc.scalar.activation(lgm[:], nm[:], AF.Ln, bias=B_COEF, scale=A_COEF)
                        outtab = l_tab0 if d == 0 else l_tab1
                        nc.sync.dma_start(
                            outtab[:, c0 * 16:(c0 + ncol) * 16],
                            lgm[:].rearrange("p a s -> p (a s)"))
                        lgms.append(lgm)
                    # single queue: Tile's DMASW sem-lane round-robin ignores
                    # queue_num, so multi-queue breaks lane/threshold
                    # semantics (sim rejects it); scatters must serialize
                    # anyway (u- and v-side rows may collide, CCE add is not
                    # atomic across queues).
                    nc.gpsimd.dma_scatter_add(
                        out_ap=s_tab[vw * win:, 0:S], in_ap=lgms[0][:],
                        idxs_ap=vs_t[:, i0:i1], num_idxs=n, num_idxs_reg=n,
                        elem_size=S, elem_step=64, queue_num=0)
                    nc.gpsimd.dma_scatter_add(
                        out_ap=s_tab[uw * win:, 0:S], in_ap=lgms[1][:],
                        idxs_ap=us_t[:, i0:i1], num_idxs=n, num_idxs_reg=n,
                        elem_size=S, elem_step=64, queue_num=0)

                for b0 in range(0, npad // 128, CH):
                    bn = min(CH, npad // 128 - b0)
                    pit2 = bpool.tile([128, CH, 64], dt.float32, tag="pit2")
                    nc.sync.dma_start(
                        pit2[:, :bn, :],
                        s_tab[:].rearrange("(b p) c -> p b c", p=128)[:, b0:b0 + bn, :])
                    cm2 = bpool.tile([128, CH, S], dt.float32, tag="cm2")
                    nc.vector.tensor_copy(out=cm2[:, :bn, :], in_=pit2[:, :bn, 0:S])
                    nc.sync.dma_start(
                        rs_in[:].rearrange("(b p) s -> p b s", p=128)[:, b0:b0 + bn, :],
                        cm2[:, :bn, :])
                nc.gpsimd.collective_compute("ReduceScatter", AL.add, replica_groups=rg,
                                             ins=[rs_in[:]], outs=[rs_out[:]])
                sv = npool.tile([128, nblk, S], dt.float32, tag="sv")
                nc.sync.dma_start(sv[:], rs_out[:].rearrange("(b p) s -> p b s", p=128))
                lb = npool.tile([128, nblk, S], dt.float32, tag="lb")
                nc.vector.tensor_tensor(lb[:], logp[:], sv[:], op=AL.add)
                mxi = npool.tile([128, nblk], dt.float32, tag="mxi")
                nc.vector.tensor_reduce(mxi[:], lb[:], axis=AX.X, op=AL.max)
                lbn = npool.tile([128, nblk, S], dt.float32, tag="lbn")
                nc.vector.scalar_tensor_tensor(
                    lbn[:], in0=lb[:], scalar=1.0,
                    in1=mxi[:].rearrange("p (b o) -> p b o", o=1).to_broadcast([128, nblk, S]),
                    op0=AL.mult, op1=AL.subtract)
                if it < DIFFUSION:
                    nc.sync.dma_start(ag_in[:].rearrange("(b p) s -> p b s", p=128), lbn[:])
                    nc.gpsimd.collective_compute("AllGather", AL.bypass, replica_groups=rg,
                                                 ins=[ag_in[:]], outs=[ag_out[:]])
                else:
                    eb = npool.tile([128, nblk, S], dt.float32, tag="eb")
                    nc.scalar.activation(eb[:], lbn[:], AF.Exp)
                    sb = npool.tile([128, nblk], dt.float32, tag="sb")
                    nc.vector.tensor_reduce(sb[:], eb[:], axis=AX.X, op=AL.add)
                    rb = npool.tile([128, nblk], dt.float32, tag="rb")
                    nc.vector.reciprocal(rb[:], sb[:])
                    bf = npool.tile([128, nblk, S], dt.float32, tag="bf")
                    nc.vector.tensor_tensor(
                        bf[:], eb[:],
                        rb[:].rearrange("p (b o) -> p b o", o=1).to_broadcast([128, nblk, S]),
                        op=AL.mult)
                    bf16 = npool.tile([128, nblk, S], dt.float16, tag="bf16")
                    nc.vector.tensor_copy(out=bf16[:], in_=bf[:])
                    nc.sync.dma_start(
                        out16[:, S:2 * S].rearrange("(b p) s -> p b s", p=128), bf16[:])
    nc.compile()
    return nc


def _make_runner(nc):
    """Cached PJRT runner: what bass_utils.run_bass_kernel_spmd does under
    axon (bass2jax.run_bass_via_pjrt), but with the traced/lowered/compiled
    executable built once and reused, and no donated zero output buffers
    (the kernel writes every output element)."""
    import jax
    import numpy as _np
    from jax.sharding import Mesh, PartitionSpec
    from jax.experimental.shard_map import shard_map
    import concourse.mybir as mybir
    from concourse.bass2jax import (_bass_exec_p, partition_id_tensor,
                                    install_neuronx_cc_hook)

    install_neuronx_cc_hook()
    partition_name = nc.partition_id_tensor.name if nc.partition_id_tensor else None
    in_names, out_names, out_avals = [], [], []
    for alloc in nc.m.functions[0].allocations:
        if not isinstance(alloc, mybir.MemoryLocationSet):
            continue
        name = alloc.memorylocations[0].name
        if alloc.kind == "ExternalInput":
            if name != partition_name:
                in_names.append(name)
        elif alloc.kind == "ExternalOutput":
            out_names.append(name)
            out_avals.append(jax.core.ShapedArray(
                tuple(alloc.tensor_shape), mybir.dt.np(alloc.dtype)))
    in_names_full = in_names + ([partition_name] if partition_name else [])

    def _body(*args):
        operands = list(args)
        if partition_name is not None:
            operands.append(partition_id_tensor())
        return tuple(_bass_exec_p.bind(
            *operands, out_avals=tuple(out_avals), in_names=tuple(in_names_full),
            out_names=tuple(out_names), lowering_input_output_aliases=(),
            sim_require_finite=True, sim_require_nnan=True, nc=nc))

    devices = jax.devices()[:NCORES]
    mesh = Mesh(_np.asarray(devices), ("core",))
    P = PartitionSpec("core")
    sharded = jax.jit(shard_map(_body, mesh=mesh, in_specs=(P,) * len(in_names),
                                out_specs=(P,) * len(out_names), check_rep=False))
    state = {}

    def run(*global_ins):
        if "compiled" not in state:
            state["compiled"] = sharded.lower(*global_ins).compile()
        try:
            outs = state["compiled"](*global_ins)
            return [np.asarray(o) for o in outs]
        except Exception:
            # one retry for transient axon-tunnel failures
            outs = state["compiled"](*global_ins)
            return [np.asarray(o) for o in outs]

    return run


def kernel(features, W, src_nodes, dst_nodes, rev_edges):
    features = np.asarray(features, np.float32)
    W = np.asarray(W, np.float32)
    src = np.asarray(src_nodes, np.int64)
    dst = np.asarray(dst_nodes, np.int64)
    rev = np.asarray(rev_edges, np.int64)
    n_nodes, feat_dim = features.shape
    E = src.shape[0] // 2
    assert np.array_equal(rev[:E], np.arange(E) + E) and \
        np.array_equal(rev[E:], np.arange(E)), "unexpected rev_edges structure"
    u = src[:E].astype(np.int64)
    v = dst[:E].astype(np.int64)

    key = (n_nodes, feat_dim, E)
    if key not in _CACHE:
        plan = _plan(u, v, n_nodes)
        nc = _build(plan, n_nodes, feat_dim)
        _CACHE[key] = (plan, nc, _make_runner(nc))
    plan, nc, run = _CACHE[key]

    win_real, win_pad, npad = _geom(n_nodes)
    shard, feat_elems, off_us, off_vs, off_w, nelem = _blob_layout(
        plan, n_nodes, feat_dim)

    # Padded row c*shard+i maps to feature row w*win_real + (local offset)
    # where windows (win_pad rows) align to whole cores (win_pad % shard == 0).
    blob = np.empty((NCORES, nelem), np.int16)
    from concurrent.futures import ThreadPoolExecutor

    def fill_core(c):
        fv = blob[c, 0:feat_elems].view(np.float16).reshape(shard, feat_dim)
        lo = c * shard
        w = lo // win_pad
        o = lo - w * win_pad
        real = max(0, min(shard, win_real - o, n_nodes - w * win_real - o))
        fv[0:real] = features[w * win_real + o:w * win_real + o + real]
        fv[real:] = 0.0
        blob[c, off_us:off_us + plan["total"]] = plan["us_wrap"][c]
        blob[c, off_vs:off_vs + plan["total"]] = plan["vs_wrap"][c]
        blob[c, off_w:off_w + feat_dim * S] = \
            W.astype(np.float16).view(np.int16).reshape(feat_dim * S)

    with ThreadPoolExecutor(NCORES) as ex:
        list(ex.map(fill_core, range(NCORES)))

    (out16,) = run(blob)
    out16 = out16.reshape(npad, 2 * S)
    priors = np.empty((n_nodes, S), np.float32)
    beliefs = np.empty((n_nodes, S), np.float32)
    for w in range(NWIN):
        lo, n = w * win_real, min(win_real, n_nodes - w * win_real)
        priors[lo:lo + n] = out16[w * win_pad:w * win_pad + n, 0:S]
        beliefs[lo:lo + n] = out16[w * win_pad:w * win_pad + n, S:2 * S]
    return priors, beliefs
